# revision 1
# baseline (speedup 1.0000x reference)
"""Trainium2 Bass kernel for nn_KalmanGraphicalModel (gnn_message_passing).

The reference runs ITERS=100 iterations of a LINEAR 3-point stencil in time:
    x <- A' x_t + B' x_{t-1} + C' x_{t+1} + Gam y_t     (edge-replicated)
Because the update is linear and gamma is small, the composed 100-step
operator is a banded convolution with numerically tiny bandwidth D (~12 for
gamma=0.01):
    x_100[t] = sum_{|d|<=D} G_d x0[t+d] + V_d y[t+d]
So the whole problem collapses to ONE banded-matmul pass on device:
  - time axis folded 16-way into the partition dim (16 blocks x 8 rows = 128)
  - the stencil taps become 128x128 block-banded weight matrices; taps that
    cross a fold boundary land in neighbor-column streams (sigma = -S..S)
  - per 512-col tile: (2S+1) x-matmuls + (2S+1) y-matmuls accumulate in PSUM
T is sharded across 8 cores; the first/last 128 columns (edge-rule
influenced + window zero-padding) are computed host-side on tiny strips.
"""
import os
import numpy as np

N, M, T, ITERS = 8, 4, 500000, 100
NCORES = 8
L = T // NCORES          # 62500 timesteps per core
FOLD = 16                # time-fold factor -> 16 blocks x 8 rows = 128 partitions
NC = 3908                # out cols per core: 16*3908 = 62528 >= 62500
EDGE = 128               # host-computed override width at the two true edges
STRIP = 384              # width of host edge strips
TAU = 1e-10              # tap truncation threshold (relative)

_PROGRAM_CACHE = {}
USE_BF16 = bool(int(os.environ.get("KALMAN_BF16", "0")))


def _compose_taps(F, H, Q, R, gamma):
    """Banded composition of the 100 linear steps, in float64."""
    Qinv = np.linalg.inv(Q)
    Rinv = np.linalg.inv(R)
    negQinv = -Qinv
    FtQinv = F.T @ Qinv
    HtRinv = H.T @ Rinv
    Z1 = np.eye(N); Z1[0, 0] = 0.0
    Z2 = np.eye(N); Z2[-1, -1] = 0.0
    Ap = np.eye(N) + gamma * (negQinv @ Z1 - FtQinv @ Z2 @ F - HtRinv @ H)
    Bp = -gamma * (negQinv @ Z1 @ F)
    Cp = gamma * (FtQinv @ Z2)
    Gam = gamma * HtRinv

    K = ITERS
    G = np.zeros((2 * K + 1, N, N))
    V = np.zeros((2 * K + 1, N, M))
    G[K] = np.eye(N)
    for _ in range(K):
        Gn = np.einsum("ij,djk->dik", Ap, G)
        Gn[:-1] += np.einsum("ij,djk->dik", Bp, G[1:])
        Gn[1:] += np.einsum("ij,djk->dik", Cp, G[:-1])
        Vn = np.einsum("ij,djk->dik", Ap, V)
        Vn[:-1] += np.einsum("ij,djk->dik", Bp, V[1:])
        Vn[1:] += np.einsum("ij,djk->dik", Cp, V[:-1])
        Vn[K] += Gam
        G, V = Gn, Vn

    gmax = np.abs(G).max(axis=(1, 2))
    vmax = np.abs(V).max(axis=(1, 2))
    scale = max(gmax.max(), vmax.max())
    keep = np.where((gmax > TAU * scale) | (vmax > TAU * scale))[0]
    D = int(max(1, np.abs(keep - K).max()))
    return G, V, D, (Ap.astype(np.float32), Bp.astype(np.float32),
                     Cp.astype(np.float32), Gam.astype(np.float32))


def _build_program(S):
    """Build + schedule the Bass/Tile program (cached per S)."""
    import concourse.bass as bass
    import concourse.tile as tile
    from concourse import bacc, mybir

    if S in _PROGRAM_CACHE:
        return _PROGRAM_CACHE[S]

    CW = NC + 2 * S
    nsig = 2 * S + 1
    f32 = mybir.dt.float32
    f32r = mybir.dt.bfloat16 if USE_BF16 else mybir.dt.float32r

    nc = bacc.Bacc("TRN2", target_bir_lowering=False, debug=False,
                   enable_asserts=False, num_devices=NCORES)
    xf = nc.dram_tensor("xf", [128, CW], f32r, kind="ExternalInput").ap()
    yf = nc.dram_tensor("yf", [64, CW], f32r, kind="ExternalInput").ap()
    wx = nc.dram_tensor("wx", [128, nsig * 128], f32r, kind="ExternalInput").ap()
    wy = nc.dram_tensor("wy", [64, nsig * 128], f32r, kind="ExternalInput").ap()
    out = nc.dram_tensor("out", [128, NC], f32, kind="ExternalOutput").ap()

    TS = 512
    tiles = []
    c = 0
    while c < NC:
        tiles.append((c, min(TS, NC - c)))
        c += TS

    with tile.TileContext(nc) as tc:
        with tc.tile_pool(name="consts", bufs=1) as consts, \
             tc.tile_pool(name="ps", bufs=8, space="PSUM") as ps_pool, \
             tc.tile_pool(name="outp", bufs=8) as outp:
            wxsb = consts.tile([128, nsig * 128], f32r)
            wysb = consts.tile([64, nsig * 128], f32r)
            # scalar (Activation HWDGE) carries weights + y; sync carries x.
            # The two direct-DMA streams run in parallel, so the first
            # accumulation group's operands land ~2us in instead of ~13us.
            nc.scalar.dma_start(wysb[:], wy[:])
            nc.scalar.dma_start(wxsb[:], wx[:])
            xsb = consts.tile([128, CW], f32r)
            ysb = consts.tile([64, CW], f32r)
            # chunked loads so compute can start before the whole window lands;
            # the first tiles' operands go in small chunks so the PE can start
            # as early as possible behind the serial per-engine DMA streams.
            dma_chunks = []
            c = 0
            while c < CW:
                cn_ = 256 if c < 1024 else 512
                dma_chunks.append((c, min(cn_, CW - c)))
                c += cn_
            for (c0, cn) in dma_chunks:
                nc.sync.dma_start(xsb[:, c0:c0 + cn], xf[:, c0:c0 + cn])
                nc.scalar.dma_start(ysb[:, c0:c0 + cn], yf[:, c0:c0 + cn])
            for ti, (c0, cn) in enumerate(tiles):
                ps = ps_pool.tile([128, cn], f32)
                for si in range(nsig):
                    nc.tensor.matmul(
                        ps[:],
                        wysb[:, si * 128:(si + 1) * 128],
                        ysb[:, c0 + si:c0 + si + cn],
                        start=(si == 0), stop=False)
                for si in range(nsig):
                    nc.tensor.matmul(
                        ps[:],
                        wxsb[:, si * 128:(si + 1) * 128],
                        xsb[:, c0 + si:c0 + si + cn],
                        start=False, stop=(si == nsig - 1))
                ot = outp.tile([128, cn], f32)
                nc.vector.tensor_copy(ot[:], ps[:])
                eng = nc.scalar if ti % 2 else nc.sync
                eng.dma_start(out[:, c0:c0 + cn], ot[:])
    nc.compile()
    _PROGRAM_CACHE[S] = nc
    return nc


def _fold(a, rows, CW):
    # a: (rows, 16*CW) -> (rows*16 partitions, CW); partition b*rows+r holds
    # times t = c*16+b
    return np.ascontiguousarray(
        a.reshape(rows, CW, FOLD).transpose(2, 0, 1).reshape(FOLD * rows, CW))


def _run_edge_strip(x0, y, Ap, Bp, Cp, Gam):
    # reference-style edge replication on both strip ends; only the true-edge
    # side of the strip is consumed, the other side's garbage stays >100 cols
    # away from the EDGE-wide region we keep.
    x = x0.copy()
    for _ in range(ITERS):
        xp = np.concatenate([x[:, :1], x[:, :-1]], axis=1)
        xf_ = np.concatenate([x[:, 1:], x[:, -1:]], axis=1)
        x = (Ap @ x + Bp @ xp + Cp @ xf_ + Gam @ y).astype(np.float32)
    return x


def kernel(xs, ys, F, H, Q, R, gamma):
    from concourse.bass_utils import run_bass_kernel_spmd

    xs = np.asarray(xs, dtype=np.float32)
    ysv = np.asarray(ys, dtype=np.float32)
    F64 = np.asarray(F, dtype=np.float64)
    H64 = np.asarray(H, dtype=np.float64)
    Q64 = np.asarray(Q, dtype=np.float64)
    R64 = np.asarray(R, dtype=np.float64)
    g = float(np.asarray(gamma))

    G, V, D, mats32 = _compose_taps(F64, H64, Q64, R64, g)
    S = (D + FOLD - 1) // FOLD
    assert S <= 7, f"bandwidth D={D} too large for single-pass kernel"
    CW = NC + 2 * S
    nsig = 2 * S + 1

    # ---- weights ----
    K = ITERS
    WX = np.zeros((nsig, 128, 128), dtype=np.float32)
    WY = np.zeros((nsig, 64, 128), dtype=np.float32)
    for si in range(nsig):
        sig = si - S
        for bo in range(FOLD):
            for bi in range(FOLD):
                d = sig * FOLD + bi - bo
                if abs(d) > D:
                    continue
                WX[si, bi * 8:bi * 8 + 8, bo * 8:bo * 8 + 8] = G[K + d].T
                WY[si, bi * 4:bi * 4 + 4, bo * 8:bo * 8 + 8] = V[K + d].T

    # ---- per-core folded input windows ----
    pad = FOLD * S
    padR = pad + (FOLD * NC - L)          # right overhang of core 7's window
    xs_p = np.zeros((N, T + pad + padR), dtype=np.float32)
    ys_p = np.zeros((M, T + pad + padR), dtype=np.float32)
    xs_p[:, pad:pad + T] = xs
    ys_p[:, pad:pad + T] = ysv
    # SBUF weight tile is (parts, nsig*128), sigma-major along columns
    wx_np = np.ascontiguousarray(WX.transpose(1, 0, 2).reshape(128, nsig * 128))
    wy_np = np.ascontiguousarray(WY.transpose(1, 0, 2).reshape(64, nsig * 128))
    in_maps = []
    for i in range(NCORES):
        o = i * L
        in_maps.append({
            "xf": _fold(xs_p[:, o:o + FOLD * CW], N, CW),
            "yf": _fold(ys_p[:, o:o + FOLD * CW], M, CW),
            "wx": wx_np,
            "wy": wy_np,
        })

    if USE_BF16:
        import ml_dtypes
        bf16 = np.dtype(ml_dtypes.bfloat16)
        for m_ in in_maps:
            for k in m_:
                m_[k] = m_[k].astype(bf16)
    nc = _build_program(S)
    trace = bool(int(os.environ.get("KALMAN_TRACE", "0")))
    res = run_bass_kernel_spmd(nc, in_maps, core_ids=list(range(NCORES)),
                               trace=trace)
    if trace and res.exec_time_ns is not None:
        print(f"HW exec time: {res.exec_time_ns} ns")
        print(f"HW exec time mean: {res.mean_exec_time_ns} ns")

    out_full = np.empty((N, T), dtype=np.float32)
    for i in range(NCORES):
        o = i * L
        Out = res.results[i]["out"]                       # (128, NC)
        unf = Out.reshape(FOLD, N, NC).transpose(1, 2, 0).reshape(N, FOLD * NC)
        out_full[:, o:o + L] = unf[:, :L]

    # ---- host edge strips (exact edge-replication dynamics) ----
    Ap32, Bp32, Cp32, Gam32 = mats32
    left = _run_edge_strip(xs[:, :STRIP], ysv[:, :STRIP],
                           Ap32, Bp32, Cp32, Gam32)
    right = _run_edge_strip(xs[:, -STRIP:], ysv[:, -STRIP:],
                            Ap32, Bp32, Cp32, Gam32)
    out_full[:, :EDGE] = left[:, :EDGE]
    out_full[:, -EDGE:] = right[:, -EDGE:]
    return out_full



# revision 3
# speedup vs baseline: 1.0859x; 1.0859x over previous
"""Trainium2 Bass kernel for nn_KalmanGraphicalModel (gnn_message_passing).

The reference runs ITERS=100 iterations of a LINEAR 3-point stencil in time:
    x <- A' x_t + B' x_{t-1} + C' x_{t+1} + Gam y_t     (edge-replicated)
The composed 100-step operator is a banded convolution with tiny bandwidth
D (=12 for gamma=0.01):
    x_100[t] = sum_{|d|<=D} G_d x0[t+d] + V_d y[t+d]
One banded-matmul pass on device:
  - time axis folded 16-way into the partition dim (16 blocks x 8 rows = 128)
  - D<=16 makes the block-band width 3 (sigma in {-1,0,+1})
  - per 512-col PSUM tile, 4 matmuls accumulate the full stencil:
      1. center x tap          (bf16, 128-contraction)
      2. both outer x taps     (fp8 DoubleRow: 2 taps in one pass via an
         overlapping strided moving AP; outer tap mass is ~2% of center
         so fp8 noise is negligible)
      3. y taps sig=-1,+1      (bf16, packed into the two partition halves:
         top 64 parts = y, bottom 64 = y shifted 2 cols)
      4. y center tap          (bf16, 64-contraction)
  - inputs/outputs bf16 (halves HBM traffic; output upcast on host)
  - DMAs spread across 4 engine queues (sync=x, scalar=y, gpsimd=weights+x8,
    vector=psum-copies+out) so dispatch latency is off the critical path
T is sharded across 8 cores; the first/last 128 columns (edge-rule
influenced + window zero-padding) are computed host-side on tiny strips.
"""
import os
import numpy as np

N, M, T, ITERS = 8, 4, 500000, 100
NCORES = 8
L = T // NCORES          # 62500 timesteps per core
FOLD = 16                # 16 blocks x 8 rows = 128 partitions
NC = 3908                # out cols per core: 16*3908 = 62528 >= 62500
CW = NC + 2              # input window cols (1-col halo each side)
EDGE = 128               # host-computed override width at the two true edges
STRIP = 384              # width of host edge strips
TS = 512                 # PSUM tile cols

_PROGRAM_CACHE = {}
USE_FP8X = bool(int(os.environ.get("KALMAN_FP8X", "1")))


def _compose_taps(F, H, Q, R, gamma):
    """Banded composition of the 100 linear steps, in float64."""
    Qinv = np.linalg.inv(Q)
    Rinv = np.linalg.inv(R)
    negQinv = -Qinv
    FtQinv = F.T @ Qinv
    HtRinv = H.T @ Rinv
    Z1 = np.eye(N); Z1[0, 0] = 0.0
    Z2 = np.eye(N); Z2[-1, -1] = 0.0
    Ap = np.eye(N) + gamma * (negQinv @ Z1 - FtQinv @ Z2 @ F - HtRinv @ H)
    Bp = -gamma * (negQinv @ Z1 @ F)
    Cp = gamma * (FtQinv @ Z2)
    Gam = gamma * HtRinv

    K = ITERS
    G = np.zeros((2 * K + 1, N, N))
    V = np.zeros((2 * K + 1, N, M))
    G[K] = np.eye(N)
    for _ in range(K):
        Gn = np.einsum("ij,djk->dik", Ap, G)
        Gn[:-1] += np.einsum("ij,djk->dik", Bp, G[1:])
        Gn[1:] += np.einsum("ij,djk->dik", Cp, G[:-1])
        Vn = np.einsum("ij,djk->dik", Ap, V)
        Vn[:-1] += np.einsum("ij,djk->dik", Bp, V[1:])
        Vn[1:] += np.einsum("ij,djk->dik", Cp, V[:-1])
        Vn[K] += Gam
        G, V = Gn, Vn

    gmax = np.abs(G).max(axis=(1, 2))
    vmax = np.abs(V).max(axis=(1, 2))
    scale = max(gmax.max(), vmax.max())
    keep = np.where((gmax > 1e-7 * scale) | (vmax > 1e-7 * scale))[0]
    D = int(max(1, np.abs(keep - K).max()))
    return G, V, D, (Ap.astype(np.float32), Bp.astype(np.float32),
                     Cp.astype(np.float32), Gam.astype(np.float32))


def _overlap_ap(tile_ap, c0, cn, cw):
    """[128, 2, cn] view of a [128, cw] tile: (p, k, c) -> tile[p, c0+2k+c]."""
    import bass_rust
    s = tile_ap[:, c0:c0 + cn]
    a = s.copy()
    ppair = tuple(s.ap[0])
    a.ap = bass_rust.VecI64Pair([ppair, (2, 2), (1, cn)])
    return a


def _build_program():
    import concourse.tile as tile
    from concourse import bacc, mybir

    key = ("v2", USE_FP8X)
    if key in _PROGRAM_CACHE:
        return _PROGRAM_CACHE[key]

    f32 = mybir.dt.float32
    bf16 = mybir.dt.bfloat16
    f8 = mybir.dt.float8e4

    nc = bacc.Bacc("TRN2", target_bir_lowering=False, debug=False,
                   enable_asserts=False, num_devices=NCORES)
    xb = nc.dram_tensor("xb", [128, CW], bf16, kind="ExternalInput").ap()
    ypk = nc.dram_tensor("ypk", [128, CW], bf16, kind="ExternalInput").ap()
    wb = nc.dram_tensor("wb", [128, 640], bf16, kind="ExternalInput").ap()
    if USE_FP8X:
        x8 = nc.dram_tensor("x8", [128, CW], f8, kind="ExternalInput").ap()
        w8 = nc.dram_tensor("w8", [128, 256], f8, kind="ExternalInput").ap()
    out = nc.dram_tensor("out", [128, NC], bf16, kind="ExternalOutput").ap()

    tiles = []
    c = 0
    while c < NC:
        tiles.append((c, min(TS, NC - c)))
        c += TS

    # input chunk splits: small first chunk so tile 0 starts early
    chunks = [(0, 520), (520, 1024), (1544, 1024), (2568, CW - 2568)]

    with tile.TileContext(nc) as tc:
        with tc.tile_pool(name="consts", bufs=1) as consts, \
             tc.tile_pool(name="ps", bufs=8, space="PSUM") as ps_pool:
            wbsb = consts.tile([128, 640], bf16)
            nc.gpsimd.dma_start(wbsb[:], wb[:])
            if USE_FP8X:
                w8sb = consts.tile([128, 256], f8)
                x8sb = consts.tile([128, CW], f8)
                nc.gpsimd.dma_start(w8sb[:], w8[:])
            xsb = consts.tile([128, CW], bf16)
            ysb = consts.tile([128, CW], bf16)
            osb = consts.tile([128, NC], bf16)
            for (c0, cn) in chunks:
                nc.sync.dma_start(xsb[:, c0:c0 + cn], xb[:, c0:c0 + cn])
                nc.scalar.dma_start(ysb[:, c0:c0 + cn], ypk[:, c0:c0 + cn])
            if USE_FP8X:
                for (c0, cn) in ((0, 520), (520, 1536), (2056, CW - 2056)):
                    nc.gpsimd.dma_start(x8sb[:, c0:c0 + cn],
                                        x8[:, c0:c0 + cn])
                w83d = w8sb[:, 0:256].rearrange("p (k m) -> p k m", k=2)

            for ti, (c0, cn) in enumerate(tiles):
                ps = ps_pool.tile([128, cn], f32)
                # center x tap (sigma=0): moving offset c0+1
                nc.tensor.matmul(ps[:], wbsb[:, 128:256],
                                 xsb[:, c0 + 1:c0 + 1 + cn],
                                 start=True, stop=False)
                if USE_FP8X:
                    # outer x taps (sigma=-1,+1) in one fp8 DoubleRow pass
                    nc.tensor.matmul(
                        ps[:], w83d, _overlap_ap(x8sb, c0, cn, CW),
                        start=False, stop=False,
                        perf_mode=mybir.MatmulPerfMode.DoubleRow)
                else:
                    nc.tensor.matmul(ps[:], wbsb[:, 0:128],
                                     xsb[:, c0:c0 + cn],
                                     start=False, stop=False)
                    nc.tensor.matmul(ps[:], wbsb[:, 256:384],
                                     xsb[:, c0 + 2:c0 + 2 + cn],
                                     start=False, stop=False)
                # y outer taps packed in partition halves
                nc.tensor.matmul(ps[:], wbsb[:, 384:512],
                                 ysb[:, c0:c0 + cn],
                                 start=False, stop=False)
                # y center tap (64-contraction)
                nc.tensor.matmul(ps[:], wbsb[0:64, 512:640],
                                 ysb[0:64, c0 + 1:c0 + 1 + cn],
                                 start=False, stop=True)
                nc.vector.tensor_copy(osb[:, c0:c0 + cn], ps[:])
                if ti % 2 == 1 or ti == len(tiles) - 1:
                    o0 = (ti // 2) * 2 * TS
                    o1 = c0 + cn
                    nc.gpsimd.dma_start(out[:, o0:o1], osb[:, o0:o1])
    nc.compile()
    _PROGRAM_CACHE[key] = nc
    return nc


def _fold(a, rows, width):
    # a: (rows, 16*width) -> (rows*16 partitions, width); partition b*rows+r
    # holds times t = c*16 + b
    return np.ascontiguousarray(
        a.reshape(rows, width, FOLD).transpose(2, 0, 1).reshape(
            FOLD * rows, width))


def _run_edge_strip(x0, y, Ap, Bp, Cp, Gam):
    x = x0.copy()
    for _ in range(ITERS):
        xp = np.concatenate([x[:, :1], x[:, :-1]], axis=1)
        xf_ = np.concatenate([x[:, 1:], x[:, -1:]], axis=1)
        x = (Ap @ x + Bp @ xp + Cp @ xf_ + Gam @ y).astype(np.float32)
    return x


def kernel(xs, ys, F, H, Q, R, gamma):
    import ml_dtypes
    from concourse.bass_utils import run_bass_kernel_spmd
    from concourse import mybir

    bf16 = np.dtype(ml_dtypes.bfloat16)
    f8np = mybir.dt.np(mybir.dt.float8e4)

    xs = np.asarray(xs, dtype=np.float32)
    ysv = np.asarray(ys, dtype=np.float32)
    g = float(np.asarray(gamma))

    G, V, D, mats32 = _compose_taps(
        np.asarray(F, np.float64), np.asarray(H, np.float64),
        np.asarray(Q, np.float64), np.asarray(R, np.float64), g)
    assert D <= FOLD, f"bandwidth D={D} too large for 3-tap kernel"
    K = ITERS

    # ---- block-banded weights: WX[si] (128x128), WY[si] (64x128), si=0..2
    WX = np.zeros((3, 128, 128), dtype=np.float32)
    WY = np.zeros((3, 64, 128), dtype=np.float32)
    for si in range(3):
        sig = si - 1
        for bo in range(FOLD):
            for bi in range(FOLD):
                d = sig * FOLD + bi - bo
                if abs(d) > D:
                    continue
                WX[si, bi * 8:bi * 8 + 8, bo * 8:bo * 8 + 8] = G[K + d].T
                WY[si, bi * 4:bi * 4 + 4, bo * 8:bo * 8 + 8] = V[K + d].T

    wb_np = np.zeros((128, 640), dtype=np.float32)
    wb_np[:, 0:128] = WX[0]
    wb_np[:, 128:256] = WX[1]
    wb_np[:, 256:384] = WX[2]
    wb_np[0:64, 384:512] = WY[0]       # ypk top half: y shift 0  -> sigma=-1
    wb_np[64:128, 384:512] = WY[2]     # ypk bottom half: y shift 2 -> sigma=+1
    wb_np[0:64, 512:640] = WY[1]       # center y tap
    wb_np = wb_np.astype(bf16)
    if USE_FP8X:
        w8_np = np.concatenate([WX[0], WX[2]], axis=1).astype(f8np)

    # ---- per-core folded input windows ----
    pad = FOLD                               # S=1 halo in timesteps
    xw = FOLD * CW                           # 62560 window timesteps
    yw = FOLD * (CW + 2)                     # 62592 (bottom half needs +2 cols)
    xs_p = np.zeros((N, 7 * L + xw), dtype=np.float32)
    ys_p = np.zeros((M, 7 * L + yw), dtype=np.float32)
    xs_p[:, pad:pad + T] = xs
    ys_p[:, pad:pad + T] = ysv
    in_maps = []
    for i in range(NCORES):
        o = i * L
        xf = _fold(xs_p[:, o:o + xw], N, CW)
        yfw = _fold(ys_p[:, o:o + yw], M, CW + 2)
        ypk_np = np.concatenate([yfw[:, 0:CW], yfw[:, 2:CW + 2]], axis=0)
        m_ = {
            "xb": xf.astype(bf16),
            "ypk": ypk_np.astype(bf16),
            "wb": wb_np,
        }
        if USE_FP8X:
            m_["x8"] = xf.astype(f8np)
            m_["w8"] = w8_np
        in_maps.append(m_)

    nc = _build_program()
    trace = bool(int(os.environ.get("KALMAN_TRACE", "0")))
    res = run_bass_kernel_spmd(nc, in_maps, core_ids=list(range(NCORES)),
                               trace=trace)
    if trace and res.exec_time_ns is not None:
        print(f"HW exec time: {res.exec_time_ns} ns")
        print(f"HW exec time mean: {res.mean_exec_time_ns} ns")

    out_full = np.empty((N, T), dtype=np.float32)
    for i in range(NCORES):
        o = i * L
        Out = np.asarray(res.results[i]["out"]).astype(np.float32)  # (128, NC)
        unf = Out.reshape(FOLD, N, NC).transpose(1, 2, 0).reshape(N, FOLD * NC)
        out_full[:, o:o + L] = unf[:, :L]

    # ---- host edge strips (exact edge-replication dynamics) ----
    Ap32, Bp32, Cp32, Gam32 = mats32
    left = _run_edge_strip(xs[:, :STRIP], ysv[:, :STRIP],
                           Ap32, Bp32, Cp32, Gam32)
    right = _run_edge_strip(xs[:, -STRIP:], ysv[:, -STRIP:],
                            Ap32, Bp32, Cp32, Gam32)
    out_full[:, :EDGE] = left[:, :EDGE]
    out_full[:, -EDGE:] = right[:, -EDGE:]
    return out_full


# revision 8
# speedup vs baseline: 1.2125x; 1.1166x over previous
"""Trainium2 Bass kernel for nn_KalmanGraphicalModel (gnn_message_passing).

The reference runs ITERS=100 iterations of a LINEAR 3-point stencil in time:
    x <- A' x_t + B' x_{t-1} + C' x_{t+1} + Gam y_t     (edge-replicated)
The composed 100-step operator is a banded convolution with tiny bandwidth
D (=12 for gamma=0.01):
    x_100[t] = sum_{|d|<=D} G_d x0[t+d] + V_d y[t+d]
One banded-matmul pass on device:
  - time axis folded 16-way into the partition dim (16 blocks x 8 rows = 128)
  - D<=16 makes the block-band width 3 (sigma in {-1,0,+1})
  - per 512-col PSUM tile, 4 matmuls accumulate the full stencil:
      1. center x tap          (bf16, 128-contraction)
      2. both outer x taps     (fp8 DoubleRow: 2 taps in one pass via an
         overlapping strided moving AP; outer tap mass is ~2% of center
         so fp8 noise is negligible)
      3. y taps sig=-1,+1      (bf16, packed into the two partition halves:
         top 64 parts = y, bottom 64 = y shifted 2 cols)
      4. y center tap          (bf16, 64-contraction)
  - inputs/outputs bf16 (halves HBM traffic; output upcast on host)
  - DMAs spread across 4 engine queues (sync=x, scalar=y, gpsimd=weights+x8,
    vector=psum-copies+out) so dispatch latency is off the critical path
T is sharded across 8 cores; the first/last 128 columns (edge-rule
influenced + window zero-padding) are computed host-side on tiny strips.
"""
import os
import numpy as np

N, M, T, ITERS = 8, 4, 500000, 100
NCORES = 8
L = T // NCORES          # 62500 timesteps per core
FOLD = 16                # 16 blocks x 8 rows = 128 partitions
NC = 3908                # out cols per core: 16*3908 = 62528 >= 62500
CW = NC + 2              # input window cols (1-col halo each side)
EDGE = 128               # host-computed override width at the two true edges
STRIP = 384              # width of host edge strips
TS = 512                 # PSUM tile cols

_PROGRAM_CACHE = {}
USE_FP8X = bool(int(os.environ.get("KALMAN_FP8X", "1")))
F8SCALE = float(os.environ.get("KALMAN_F8SCALE", "1"))


def _compose_taps(F, H, Q, R, gamma):
    """Banded composition of the 100 linear steps, in float64."""
    Qinv = np.linalg.inv(Q)
    Rinv = np.linalg.inv(R)
    negQinv = -Qinv
    FtQinv = F.T @ Qinv
    HtRinv = H.T @ Rinv
    Z1 = np.eye(N); Z1[0, 0] = 0.0
    Z2 = np.eye(N); Z2[-1, -1] = 0.0
    Ap = np.eye(N) + gamma * (negQinv @ Z1 - FtQinv @ Z2 @ F - HtRinv @ H)
    Bp = -gamma * (negQinv @ Z1 @ F)
    Cp = gamma * (FtQinv @ Z2)
    Gam = gamma * HtRinv

    K = ITERS
    G = np.zeros((2 * K + 1, N, N))
    V = np.zeros((2 * K + 1, N, M))
    G[K] = np.eye(N)
    for _ in range(K):
        Gn = np.einsum("ij,djk->dik", Ap, G)
        Gn[:-1] += np.einsum("ij,djk->dik", Bp, G[1:])
        Gn[1:] += np.einsum("ij,djk->dik", Cp, G[:-1])
        Vn = np.einsum("ij,djk->dik", Ap, V)
        Vn[:-1] += np.einsum("ij,djk->dik", Bp, V[1:])
        Vn[1:] += np.einsum("ij,djk->dik", Cp, V[:-1])
        Vn[K] += Gam
        G, V = Gn, Vn

    gmax = np.abs(G).max(axis=(1, 2))
    vmax = np.abs(V).max(axis=(1, 2))
    scale = max(gmax.max(), vmax.max())
    keep = np.where((gmax > 1e-7 * scale) | (vmax > 1e-7 * scale))[0]
    D = int(max(1, np.abs(keep - K).max()))
    return G, V, D, (Ap.astype(np.float32), Bp.astype(np.float32),
                     Cp.astype(np.float32), Gam.astype(np.float32))


def _overlap_ap(tile_ap, c0, cn, cw):
    """[128, 2, cn] view of a [128, cw] tile: (p, k, c) -> tile[p, c0+2k+c]."""
    import bass_rust
    s = tile_ap[:, c0:c0 + cn]
    a = s.copy()
    ppair = tuple(s.ap[0])
    a.ap = bass_rust.VecI64Pair([ppair, (2, 2), (1, cn)])
    return a


def _build_program():
    import concourse.tile as tile
    from concourse import bacc, mybir

    key = ("v2", USE_FP8X)
    if key in _PROGRAM_CACHE:
        return _PROGRAM_CACHE[key]

    f32 = mybir.dt.float32
    bf16 = mybir.dt.bfloat16
    f8 = mybir.dt.float8e4

    nc = bacc.Bacc("TRN2", target_bir_lowering=False, debug=False,
                   enable_asserts=False, num_devices=NCORES)
    xb = nc.dram_tensor("xb", [128, CW], bf16, kind="ExternalInput").ap()
    ypk = nc.dram_tensor("ypk", [128, CW], bf16, kind="ExternalInput").ap()
    wb = nc.dram_tensor("wb", [128, 640], bf16, kind="ExternalInput").ap()
    if USE_FP8X:
        x8 = nc.dram_tensor("x8", [128, CW], f8, kind="ExternalInput").ap()
        w8 = nc.dram_tensor("w8", [128, 256], f8, kind="ExternalInput").ap()
    out = nc.dram_tensor("out", [128, NC], bf16, kind="ExternalOutput").ap()

    tiles = []
    c = 0
    while c < NC:
        tiles.append((c, min(TS, NC - c)))
        c += TS

    # per-tile input chunks; tile k touches cols [k*TS, (k+1)*TS+2), so the
    # first chunk is TS+2 wide and the rest shift by TS: tile k then depends
    # on chunks 0..k only.
    bounds = [0, TS + 2]
    while bounds[-1] + TS < CW:
        bounds.append(bounds[-1] + TS)
    bounds.append(CW)
    chunks = [(bounds[i], bounds[i + 1] - bounds[i])
              for i in range(len(bounds) - 1)]

    with tile.TileContext(nc) as tc:
        with tc.tile_pool(name="consts", bufs=1) as consts, \
             tc.tile_pool(name="ps", bufs=8, space="PSUM") as ps_pool:
            wbsb = consts.tile([128, 640], bf16)
            if USE_FP8X:
                w8sb = consts.tile([128, 256], f8)
                x8sb = consts.tile([128, CW], f8)
            xsb = consts.tile([128, CW], bf16)
            ysb = consts.tile([128, CW], bf16)
            osb = consts.tile([128, NC], bf16)
            # weights first (small, unblock tile 0), then per-tile chunks:
            # sync carries x, scalar carries packed-y (+w8), gpsimd carries
            # the fp8 x copy and the output writes.
            nc.sync.dma_start(wbsb[:], wb[:])
            if USE_FP8X:
                nc.scalar.dma_start(w8sb[:], w8[:])
            for (c0, cn) in chunks:
                nc.sync.dma_start(xsb[:, c0:c0 + cn], xb[:, c0:c0 + cn])
                nc.scalar.dma_start(ysb[:, c0:c0 + cn], ypk[:, c0:c0 + cn])
            if USE_FP8X:
                for k in range(0, len(chunks), 2):
                    c0 = chunks[k][0]
                    cn = (chunks[k][1] + chunks[k + 1][1]
                          if k + 1 < len(chunks) else chunks[k][1])
                    nc.gpsimd.dma_start(x8sb[:, c0:c0 + cn],
                                        x8[:, c0:c0 + cn])
                w83d = w8sb[:, 0:256].rearrange("p (k m) -> p k m", k=2)

            for ti, (c0, cn) in enumerate(tiles):
                ps = ps_pool.tile([128, cn], f32)
                # center x tap (sigma=0): moving offset c0+1
                nc.tensor.matmul(ps[:], wbsb[:, 128:256],
                                 xsb[:, c0 + 1:c0 + 1 + cn],
                                 start=True, stop=False)
                if USE_FP8X:
                    # outer x taps (sigma=-1,+1) in one fp8 DoubleRow pass
                    nc.tensor.matmul(
                        ps[:], w83d, _overlap_ap(x8sb, c0, cn, CW),
                        start=False, stop=False,
                        perf_mode=mybir.MatmulPerfMode.DoubleRow)
                else:
                    nc.tensor.matmul(ps[:], wbsb[:, 0:128],
                                     xsb[:, c0:c0 + cn],
                                     start=False, stop=False)
                    nc.tensor.matmul(ps[:], wbsb[:, 256:384],
                                     xsb[:, c0 + 2:c0 + 2 + cn],
                                     start=False, stop=False)
                # y outer taps packed in partition halves
                nc.tensor.matmul(ps[:], wbsb[:, 384:512],
                                 ysb[:, c0:c0 + cn],
                                 start=False, stop=False)
                # y center tap (64-contraction)
                nc.tensor.matmul(ps[:], wbsb[0:64, 512:640],
                                 ysb[0:64, c0 + 1:c0 + 1 + cn],
                                 start=False, stop=True)
                nc.vector.tensor_copy(osb[:, c0:c0 + cn], ps[:])
                if ti % 2 == 1 or ti == len(tiles) - 1:
                    o0 = (ti // 2) * 2 * TS
                    o1 = c0 + cn
                    nc.gpsimd.dma_start(out[:, o0:o1], osb[:, o0:o1])
    nc.compile()
    _PROGRAM_CACHE[key] = nc
    return nc


def _fold(a, rows, width):
    # a: (rows, 16*width) -> (rows*16 partitions, width); partition b*rows+r
    # holds times t = c*16 + b
    return np.ascontiguousarray(
        a.reshape(rows, width, FOLD).transpose(2, 0, 1).reshape(
            FOLD * rows, width))


def _run_edge_strip(x0, y, Ap, Bp, Cp, Gam):
    x = x0.copy()
    for _ in range(ITERS):
        xp = np.concatenate([x[:, :1], x[:, :-1]], axis=1)
        xf_ = np.concatenate([x[:, 1:], x[:, -1:]], axis=1)
        x = (Ap @ x + Bp @ xp + Cp @ xf_ + Gam @ y).astype(np.float32)
    return x


def kernel(xs, ys, F, H, Q, R, gamma):
    import ml_dtypes
    from concourse.bass_utils import run_bass_kernel_spmd
    from concourse import mybir

    bf16 = np.dtype(ml_dtypes.bfloat16)
    f8np = mybir.dt.np(mybir.dt.float8e4)

    xs = np.asarray(xs, dtype=np.float32)
    ysv = np.asarray(ys, dtype=np.float32)
    g = float(np.asarray(gamma))

    G, V, D, mats32 = _compose_taps(
        np.asarray(F, np.float64), np.asarray(H, np.float64),
        np.asarray(Q, np.float64), np.asarray(R, np.float64), g)
    assert D <= FOLD, f"bandwidth D={D} too large for 3-tap kernel"
    K = ITERS

    # ---- block-banded weights: WX[si] (128x128), WY[si] (64x128), si=0..2
    WX = np.zeros((3, 128, 128), dtype=np.float32)
    WY = np.zeros((3, 64, 128), dtype=np.float32)
    for si in range(3):
        sig = si - 1
        for bo in range(FOLD):
            for bi in range(FOLD):
                d = sig * FOLD + bi - bo
                if abs(d) > D:
                    continue
                WX[si, bi * 8:bi * 8 + 8, bo * 8:bo * 8 + 8] = G[K + d].T
                WY[si, bi * 4:bi * 4 + 4, bo * 8:bo * 8 + 8] = V[K + d].T

    wb_np = np.zeros((128, 640), dtype=np.float32)
    wb_np[:, 0:128] = WX[0]
    wb_np[:, 128:256] = WX[1]
    wb_np[:, 256:384] = WX[2]
    wb_np[0:64, 384:512] = WY[0]       # ypk top half: y shift 0  -> sigma=-1
    wb_np[64:128, 384:512] = WY[2]     # ypk bottom half: y shift 2 -> sigma=+1
    wb_np[0:64, 512:640] = WY[1]       # center y tap
    wb_np = wb_np.astype(bf16)
    if USE_FP8X:
        w8_np = (np.concatenate([WX[0], WX[2]], axis=1)
                 * F8SCALE).astype(f8np)

    # ---- per-core folded input windows ----
    pad = FOLD                               # S=1 halo in timesteps
    xw = FOLD * CW                           # 62560 window timesteps
    yw = FOLD * (CW + 2)                     # 62592 (bottom half needs +2 cols)
    xs_p = np.zeros((N, 7 * L + xw), dtype=np.float32)
    ys_p = np.zeros((M, 7 * L + yw), dtype=np.float32)
    xs_p[:, pad:pad + T] = xs
    ys_p[:, pad:pad + T] = ysv
    in_maps = []
    for i in range(NCORES):
        o = i * L
        xf = _fold(xs_p[:, o:o + xw], N, CW)
        yfw = _fold(ys_p[:, o:o + yw], M, CW + 2)
        ypk_np = np.concatenate([yfw[:, 0:CW], yfw[:, 2:CW + 2]], axis=0)
        m_ = {
            "xb": xf.astype(bf16),
            "ypk": ypk_np.astype(bf16),
            "wb": wb_np,
        }
        if USE_FP8X:
            m_["x8"] = (xf / F8SCALE).astype(f8np) if F8SCALE != 1 \
                else xf.astype(f8np)
            m_["w8"] = w8_np
        in_maps.append(m_)

    nc = _build_program()
    trace = bool(int(os.environ.get("KALMAN_TRACE", "0")))
    res = run_bass_kernel_spmd(nc, in_maps, core_ids=list(range(NCORES)),
                               trace=trace)
    if trace and res.exec_time_ns is not None:
        print(f"HW exec time: {res.exec_time_ns} ns")
        print(f"HW exec time mean: {res.mean_exec_time_ns} ns")

    out_full = np.empty((N, T), dtype=np.float32)
    for i in range(NCORES):
        o = i * L
        Out = np.asarray(res.results[i]["out"]).astype(np.float32)  # (128, NC)
        unf = Out.reshape(FOLD, N, NC).transpose(1, 2, 0).reshape(N, FOLD * NC)
        out_full[:, o:o + L] = unf[:, :L]

    # ---- host edge strips (exact edge-replication dynamics) ----
    Ap32, Bp32, Cp32, Gam32 = mats32
    left = _run_edge_strip(xs[:, :STRIP], ysv[:, :STRIP],
                           Ap32, Bp32, Cp32, Gam32)
    right = _run_edge_strip(xs[:, -STRIP:], ysv[:, -STRIP:],
                            Ap32, Bp32, Cp32, Gam32)
    out_full[:, :EDGE] = left[:, :EDGE]
    out_full[:, -EDGE:] = right[:, -EDGE:]
    return out_full


# revision 10
# speedup vs baseline: 1.4328x; 1.1817x over previous
"""Trainium2 Bass kernel for nn_KalmanGraphicalModel (gnn_message_passing).

The reference runs ITERS=100 iterations of a LINEAR 3-point stencil in time:
    x <- A' x_t + B' x_{t-1} + C' x_{t+1} + Gam y_t     (edge-replicated)
The composed 100-step operator is a banded convolution with tiny bandwidth
D (<=8 at 1e-5 relative truncation for gamma=0.01):
    x_100[t] = sum_{|d|<=D} G_d x0[t+d] + V_d y[t+d]
One banded-matmul pass on device, 3 column-passes per output tile:
  - time axis folded 16-way into the partition dim (16 blocks x 8 rows = 128)
  - block-band sigma in {-1,0,+1}; with D<=8 the sigma=-1 block matrix only
    has nonzero contraction rows in fold-blocks 8..15 (partitions 64..127)
    and sigma=+1 only in fold-blocks 0..7 (partitions 0..63), so BOTH outer
    x taps pack into ONE matmul against a half-shifted x copy (xpk: top half
    pre-shifted 2 cols).  Same for y: center tap (64 rows) + the two outer
    blocks (32 disjoint rows each) pack into ONE 128-contraction matmul
    against yq = [y shift1; y[0:32] shift2; y[32:64] shift0].
  - per 512-col PSUM tile: center-x + packed-outer-x + packed-y, all bf16
  - inputs/outputs bf16 (output upcast on host)
  - DMAs on 3 queues (sync=wb+x, scalar=yq, gpsimd=xpk+out), per-tile
    chunks so the matmul chain starts as soon as tile 0's operands land
T is sharded across 8 cores; the first/last 128 columns (edge-rule
influenced + window zero-padding) are computed host-side on tiny strips.
"""
import os
import numpy as np

N, M, T, ITERS = 8, 4, 500000, 100
NCORES = 8
L = T // NCORES          # 62500 timesteps per core
FOLD = 16                # 16 blocks x 8 rows = 128 partitions
NC = 3908                # out cols per core: 16*3908 = 62528 >= 62500
CW = NC + 2              # input window cols (1-col halo each side)
EDGE = 128               # host-computed override width at the two true edges
STRIP = 384              # width of host edge strips
TS = 512                 # PSUM tile cols
DMAX = 8                 # tap truncation: |d|<=8 keeps the outer blocks in
                         # disjoint partition halves (tap d=9 is ~2e-6 rel)

_PROGRAM_CACHE = {}


def _compose_taps(F, H, Q, R, gamma):
    """Banded composition of the 100 linear steps, in float64."""
    Qinv = np.linalg.inv(Q)
    Rinv = np.linalg.inv(R)
    negQinv = -Qinv
    FtQinv = F.T @ Qinv
    HtRinv = H.T @ Rinv
    Z1 = np.eye(N); Z1[0, 0] = 0.0
    Z2 = np.eye(N); Z2[-1, -1] = 0.0
    Ap = np.eye(N) + gamma * (negQinv @ Z1 - FtQinv @ Z2 @ F - HtRinv @ H)
    Bp = -gamma * (negQinv @ Z1 @ F)
    Cp = gamma * (FtQinv @ Z2)
    Gam = gamma * HtRinv

    K = ITERS
    G = np.zeros((2 * K + 1, N, N))
    V = np.zeros((2 * K + 1, N, M))
    G[K] = np.eye(N)
    for _ in range(K):
        Gn = np.einsum("ij,djk->dik", Ap, G)
        Gn[:-1] += np.einsum("ij,djk->dik", Bp, G[1:])
        Gn[1:] += np.einsum("ij,djk->dik", Cp, G[:-1])
        Vn = np.einsum("ij,djk->dik", Ap, V)
        Vn[:-1] += np.einsum("ij,djk->dik", Bp, V[1:])
        Vn[1:] += np.einsum("ij,djk->dik", Cp, V[:-1])
        Vn[K] += Gam
        G, V = Gn, Vn
    return G, V, (Ap.astype(np.float32), Bp.astype(np.float32),
                  Cp.astype(np.float32), Gam.astype(np.float32))


def _build_program():
    import concourse.tile as tile
    from concourse import bacc, mybir

    key = "v4"
    if key in _PROGRAM_CACHE:
        return _PROGRAM_CACHE[key]

    f32 = mybir.dt.float32
    bf16 = mybir.dt.bfloat16

    nc = bacc.Bacc("TRN2", target_bir_lowering=False, debug=False,
                   enable_asserts=False, num_devices=NCORES)
    xb = nc.dram_tensor("xb", [128, CW], bf16, kind="ExternalInput").ap()
    xpk = nc.dram_tensor("xpk", [128, CW], bf16, kind="ExternalInput").ap()
    yq = nc.dram_tensor("yq", [128, CW], bf16, kind="ExternalInput").ap()
    wb = nc.dram_tensor("wb", [128, 384], bf16, kind="ExternalInput").ap()
    out = nc.dram_tensor("out", [128, NC], bf16, kind="ExternalOutput").ap()

    tiles = []
    c = 0
    while c < NC:
        tiles.append((c, min(TS, NC - c)))
        c += TS

    # per-tile input chunks; tile k touches cols [k*TS, (k+1)*TS+2), so the
    # first chunk is TS+2 wide and the rest shift by TS: tile k then depends
    # on chunks 0..k only.
    bounds = [0, TS + 2]
    while bounds[-1] + TS < CW:
        bounds.append(bounds[-1] + TS)
    bounds.append(CW)
    chunks = [(bounds[i], bounds[i + 1] - bounds[i])
              for i in range(len(bounds) - 1)]

    with tile.TileContext(nc) as tc:
        with tc.tile_pool(name="consts", bufs=1) as consts, \
             tc.tile_pool(name="ps", bufs=8, space="PSUM") as ps_pool:
            wbsb = consts.tile([128, 384], bf16)
            xsb = consts.tile([128, CW], bf16)
            xpsb = consts.tile([128, CW], bf16)
            ysb = consts.tile([128, CW], bf16)
            osb = consts.tile([128, NC], bf16)
            nc.sync.dma_start(wbsb[:], wb[:])
            for (c0, cn) in chunks:
                nc.sync.dma_start(xsb[:, c0:c0 + cn], xb[:, c0:c0 + cn])
                nc.scalar.dma_start(ysb[:, c0:c0 + cn], yq[:, c0:c0 + cn])
            for k in range(0, len(chunks), 2):
                c0 = chunks[k][0]
                cn = (chunks[k][1] + chunks[k + 1][1]
                      if k + 1 < len(chunks) else chunks[k][1])
                nc.gpsimd.dma_start(xpsb[:, c0:c0 + cn], xpk[:, c0:c0 + cn])

            ndone = 0
            for ti, (c0, cn) in enumerate(tiles):
                ps = ps_pool.tile([128, cn], f32)
                # center x tap (sigma=0): moving offset c0+1
                nc.tensor.matmul(ps[:], wbsb[:, 0:128],
                                 xsb[:, c0 + 1:c0 + 1 + cn],
                                 start=True, stop=False)
                # both outer x taps in one pass against half-shifted x
                nc.tensor.matmul(ps[:], wbsb[:, 128:256],
                                 xpsb[:, c0:c0 + cn],
                                 start=False, stop=False)
                # all three y taps in one pass
                nc.tensor.matmul(ps[:], wbsb[:, 256:384],
                                 ysb[:, c0:c0 + cn],
                                 start=False, stop=True)
                nc.vector.tensor_copy(osb[:, c0:c0 + cn], ps[:])
                # drain finished output columns: after odd tiles and the
                # last three tiles, alternating scalar/gpsimd
                if ti % 2 == 1 or ti >= len(tiles) - 2:
                    o0, o1 = ndone, c0 + cn
                    ndone = o1
                    eng = nc.scalar if ti % 2 else nc.gpsimd
                    eng.dma_start(out[:, o0:o1], osb[:, o0:o1])
    nc.compile()
    _PROGRAM_CACHE[key] = nc
    return nc


def _fold(a, rows, width):
    # a: (rows, 16*width) -> (rows*16 partitions, width); partition b*rows+r
    # holds times t = c*16 + b
    return np.ascontiguousarray(
        a.reshape(rows, width, FOLD).transpose(2, 0, 1).reshape(
            FOLD * rows, width))


def _run_edge_strip(x0, y, Ap, Bp, Cp, Gam):
    x = x0.copy()
    for _ in range(ITERS):
        xp = np.concatenate([x[:, :1], x[:, :-1]], axis=1)
        xf_ = np.concatenate([x[:, 1:], x[:, -1:]], axis=1)
        x = (Ap @ x + Bp @ xp + Cp @ xf_ + Gam @ y).astype(np.float32)
    return x


def kernel(xs, ys, F, H, Q, R, gamma):
    import ml_dtypes
    from concourse.bass_utils import run_bass_kernel_spmd

    bf16 = np.dtype(ml_dtypes.bfloat16)

    xs = np.asarray(xs, dtype=np.float32)
    ysv = np.asarray(ys, dtype=np.float32)
    g = float(np.asarray(gamma))

    G, V, mats32 = _compose_taps(
        np.asarray(F, np.float64), np.asarray(H, np.float64),
        np.asarray(Q, np.float64), np.asarray(R, np.float64), g)
    K = ITERS
    D = DMAX
    # sanity: dropped taps must be tiny relative to the kept mass
    drop = max(np.abs(G[K + D + 1:K + 2 * D]).max(initial=0),
               np.abs(G[K - 2 * D:K - D]).max(initial=0))
    assert drop < 1e-4 * np.abs(G).max(), f"tap truncation too lossy: {drop}"

    # ---- block-banded weights, sigma in {-1,0,+1} == si in {0,1,2} ----
    WX = np.zeros((3, 128, 128), dtype=np.float32)
    WY = np.zeros((3, 64, 128), dtype=np.float32)
    for si in range(3):
        sig = si - 1
        for bo in range(FOLD):
            for bi in range(FOLD):
                d = sig * FOLD + bi - bo
                if abs(d) > D:
                    continue
                WX[si, bi * 8:bi * 8 + 8, bo * 8:bo * 8 + 8] = G[K + d].T
                WY[si, bi * 4:bi * 4 + 4, bo * 8:bo * 8 + 8] = V[K + d].T
    # D<=8 guarantees the outer blocks live in disjoint partition halves
    assert not WX[0][:64].any() and not WX[2][64:].any()
    assert not WY[0][:32].any() and not WY[2][32:].any()

    wb_np = np.zeros((128, 384), dtype=np.float32)
    wb_np[:, 0:128] = WX[1]
    # packed outer-x stationary: rows 0:64 pair with x shift +2 (sigma=+1),
    # rows 64:128 with x shift 0 (sigma=-1)
    wb_np[0:64, 128:256] = WX[2][:64]
    wb_np[64:128, 128:256] = WX[0][64:]
    # packed y stationary: rows 0:64 = center tap (y shift 1), rows 64:96 =
    # sigma=+1 block rows (y[0:32] shift 2), rows 96:128 = sigma=-1 block
    # rows (y[32:64] shift 0)
    wb_np[0:64, 256:384] = WY[1]
    wb_np[64:96, 256:384] = WY[2][:32]
    wb_np[96:128, 256:384] = WY[0][32:]
    wb_np = wb_np.astype(bf16)

    # ---- per-core folded input windows ----
    pad = FOLD                               # S=1 halo in timesteps
    xw = FOLD * (CW + 2)
    xs_p = np.zeros((N, 7 * L + xw), dtype=np.float32)
    ys_p = np.zeros((M, 7 * L + xw), dtype=np.float32)
    xs_p[:, pad:pad + T] = xs
    ys_p[:, pad:pad + T] = ysv
    in_maps = []
    for i in range(NCORES):
        o = i * L
        xf = _fold(xs_p[:, o:o + xw], N, CW + 2)
        yf = _fold(ys_p[:, o:o + xw], M, CW + 2)
        xpk_np = np.concatenate([xf[0:64, 2:CW + 2], xf[64:128, 0:CW]],
                                axis=0)
        yq_np = np.concatenate([yf[:, 1:CW + 1], yf[0:32, 2:CW + 2],
                                yf[32:64, 0:CW]], axis=0)
        in_maps.append({
            "xb": np.ascontiguousarray(xf[:, 0:CW]).astype(bf16),
            "xpk": xpk_np.astype(bf16),
            "yq": yq_np.astype(bf16),
            "wb": wb_np,
        })

    nc = _build_program()
    trace = bool(int(os.environ.get("KALMAN_TRACE", "0")))
    res = run_bass_kernel_spmd(nc, in_maps, core_ids=list(range(NCORES)),
                               trace=trace)
    if trace and res.exec_time_ns is not None:
        print(f"HW exec time: {res.exec_time_ns} ns")
        print(f"HW exec time mean: {res.mean_exec_time_ns} ns")

    out_full = np.empty((N, T), dtype=np.float32)
    for i in range(NCORES):
        o = i * L
        Out = np.asarray(res.results[i]["out"]).astype(np.float32)  # (128,NC)
        unf = Out.reshape(FOLD, N, NC).transpose(1, 2, 0).reshape(N, FOLD * NC)
        out_full[:, o:o + L] = unf[:, :L]

    # ---- host edge strips (exact edge-replication dynamics) ----
    Ap32, Bp32, Cp32, Gam32 = mats32
    left = _run_edge_strip(xs[:, :STRIP], ysv[:, :STRIP],
                           Ap32, Bp32, Cp32, Gam32)
    right = _run_edge_strip(xs[:, -STRIP:], ysv[:, -STRIP:],
                            Ap32, Bp32, Cp32, Gam32)
    out_full[:, :EDGE] = left[:, :EDGE]
    out_full[:, -EDGE:] = right[:, -EDGE:]
    return out_full
